# revision 1
# baseline (speedup 1.0000x reference)
"""DocGCN (span-extract + 3-layer GAT + doc pooling) Trainium2 Bass kernel.

Strategy: the graph is block-diagonal over 8 documents (1024 nodes each, all
edges in-block), so each NeuronCore handles one document end to end; no
collectives. Per core:
  - span extraction + query features as one matmul per sentence
    (lhsT = features tiles, rhs = [sel_mask/cnt | qmask/8])
  - GAT layer: z = hT.T @ W (bf16); el/er logit rows via precomputed W@al,
    W@ar; dense logits ES[src,dst] = el x 1 + 1 x er as a K=2 matmul;
    P = M * max(exp(ES), exp(0.2*ES))  (= M * exp(leaky_relu(ES)));
    colsum by ones-matmul (broadcast over partitions for free);
    agg hT_next[h,dst] = z.T @ P directly in feature-major layout;
    elu(y) = min(exp(y)-1, relu(y)).
  - final: per-doc mean, qf, one fused abs-sum partition reduce.
All PE operands bf16 (validated: 9.6e-5 max rel err vs f32 reference);
PSUM/activations f32.
"""

import numpy as np

SPD = 8          # sentences per doc
L = 512          # tokens per sentence
H = 768          # hidden
SEL = 128        # selected spans (graph nodes) per sentence
NPD = SPD * SEL  # nodes per doc = 1024
KL = L // 128    # 4 L-chunks
KH = H // 128    # 6 H-chunks
NCH = NPD // 128  # 8 node chunks
SW = 136         # smask width: 128 sel cols + qmask col + pad
D = 8            # docs = cores
NEG = 0.2

_PROG = {}


def _ensure_env():
    import sys, types
    for p in ("/opt/trn_rl_repo", "/opt/trn_rl_repo/concourse"):
        if p not in sys.path:
            sys.path.insert(0, p)
    if "antenv.axon_hooks" not in sys.modules:
        try:
            import antenv
            mod = types.ModuleType("antenv.axon_hooks")
            mod._hook = None
            mod.set_axon_ntff_profile_hook = lambda h: setattr(mod, "_hook", h)
            mod.get_axon_ntff_profile_hook = lambda: mod._hook
            sys.modules["antenv.axon_hooks"] = mod
            antenv.axon_hooks = mod
            if "/root/.axon_site" not in sys.path:
                sys.path.insert(0, "/root/.axon_site")
            from trn_agent_boot import trn_boot
            h = trn_boot._ntff_profile_via_ctypes("/opt/axon/libaxon_pjrt.so")
            if h is not None:
                mod.set_axon_ntff_profile_hook(h)
        except Exception:
            pass


def _build_program(debug=False):
    import concourse.bacc as bacc
    import concourse.tile as tile
    from concourse import mybir
    from contextlib import ExitStack

    f32 = mybir.dt.float32
    bf16 = mybir.dt.bfloat16
    AF = mybir.ActivationFunctionType
    OP = mybir.AluOpType
    AX = mybir.AxisListType

    nc = bacc.Bacc(None, target_bir_lowering=False)

    feats = nc.dram_tensor("feats", [SPD, 128, KL, H], bf16, kind="ExternalInput")
    smask = nc.dram_tensor("smask", [SPD, 128, KL, SW], bf16, kind="ExternalInput")
    mmask = nc.dram_tensor("mmask", [NCH, 128, NPD], bf16, kind="ExternalInput")
    Ws, wals, wars = [], [], []
    for i in range(3):
        Ws.append(nc.dram_tensor(f"W{i}", [128, KH, H], bf16, kind="ExternalInput"))
        wals.append(nc.dram_tensor(f"wal{i}", [128, KH], bf16, kind="ExternalInput"))
        wars.append(nc.dram_tensor(f"war{i}", [128, KH], bf16, kind="ExternalInput"))
    out_d = nc.dram_tensor("out", [1, 1], f32, kind="ExternalOutput")
    if debug:
        dbg_h0 = nc.dram_tensor("dbg_h0", [KH, 128, NPD], bf16, kind="ExternalOutput")
        dbg_qf = nc.dram_tensor("dbg_qf", [KH, 128, SPD], f32, kind="ExternalOutput")
        dbg_z = nc.dram_tensor("dbg_z", [128, NCH * H], bf16, kind="ExternalOutput")
        dbg_el = nc.dram_tensor("dbg_el", [2, NPD], bf16, kind="ExternalOutput")
        dbg_P = nc.dram_tensor("dbg_P", [NCH, 128, 512], bf16, kind="ExternalOutput")
        dbg_cs = nc.dram_tensor("dbg_cs", [128, 512], f32, kind="ExternalOutput")
        dbg_h1 = nc.dram_tensor("dbg_h1", [KH, 128, NPD], bf16, kind="ExternalOutput")

    with tile.TileContext(nc) as tc:
        with ExitStack() as ctx:
            const = ctx.enter_context(tc.tile_pool(name="const", bufs=1))
            wpool = ctx.enter_context(tc.tile_pool(name="wpool", bufs=2))
            fpool = ctx.enter_context(tc.tile_pool(name="fpool", bufs=2))
            spool = ctx.enter_context(tc.tile_pool(name="spool", bufs=2))
            abpool = ctx.enter_context(tc.tile_pool(name="abpool", bufs=6))
            ppool = ctx.enter_context(tc.tile_pool(name="ppool", bufs=10))
            npool = ctx.enter_context(tc.tile_pool(name="npool", bufs=6))
            ps = ctx.enter_context(tc.tile_pool(name="ps", bufs=8, space="PSUM"))

            ones128 = const.tile([128, 128], bf16, name="ones128", tag="ones128")
            nc.vector.memset(ones128[:], 1.0)

            # M mask tiles (loaded once, used all 3 layers)
            mm = []
            for c in range(NCH):
                t = const.tile([128, NPD], bf16, name=f"mm{c}", tag=f"mm{c}")
                nc.sync.dma_start(out=t[:], in_=mmask[c])
                mm.append(t)

            # hT double buffer: hset[0] = layer input, hset[1] = layer output
            hset = [[const.tile([128, NPD], bf16, name=f"h{p}_{m}", tag=f"h{p}_{m}") for m in range(KH)]
                    for p in range(2)]
            # z (node-major) [128, NCH*H] bf16
            z = const.tile([128, NCH * H], bf16, name="z", tag="z")
            # logit rows
            # logit rows (single-partition tiles; ES built by two K=1 matmuls)
            elrow = const.tile([1, NPD], bf16, name="elrow", tag="elrow")
            errow = const.tile([1, NPD], bf16, name="errow", tag="errow")
            onesrow = const.tile([1, NPD], bf16, name="onesrow", tag="onesrow")
            nc.vector.memset(onesrow[:], 1.0)
            # qf accumulators
            qfacc = [const.tile([128, SPD], f32, name=f"qf{m}", tag=f"qf{m}") for m in range(KH)]
            dfin = const.tile([128, KH], f32, name="dfin", tag="dfin")

            # ---------------- span extraction + qf ----------------
            h0 = hset[0]
            for s in range(SPD):
                ft = fpool.tile([128, KL, H], bf16, name="ft", tag="ft")
                nc.sync.dma_start(out=ft[:], in_=feats[s])
                st = spool.tile([128, KL, SW], bf16, name="st", tag="st")
                nc.sync.dma_start(out=st[:], in_=smask[s])
                for m in range(KH):
                    p = ps.tile([128, 512], f32, name="ps", tag="ps")
                    for k in range(KL):
                        nc.tensor.matmul(
                            p[:, 0:SW],
                            ft[:, k, m * 128:(m + 1) * 128],
                            st[:, k, :],
                            start=(k == 0), stop=(k == KL - 1),
                        )
                    nc.vector.tensor_copy(h0[m][:, s * 128:(s + 1) * 128], p[:, 0:128])
                    nc.vector.tensor_copy(qfacc[m][:, s:s + 1], p[:, 128:129])

            if debug:
                for m in range(KH):
                    nc.sync.dma_start(out=dbg_h0[m], in_=h0[m][:])
                    nc.sync.dma_start(out=dbg_qf[m], in_=qfacc[m][:])

            # ---------------- GAT layers ----------------
            for li in range(3):
                hin = hset[li % 2]
                hout = hset[(li + 1) % 2]
                Wt = wpool.tile([128, KH, H], bf16, name="W", tag="W")
                nc.sync.dma_start(out=Wt[:], in_=Ws[li][:])
                walT = wpool.tile([128, KH], bf16, name="wal", tag="wal")
                nc.sync.dma_start(out=walT[:], in_=wals[li][:])
                warT = wpool.tile([128, KH], bf16, name="war", tag="war")
                nc.sync.dma_start(out=warT[:], in_=wars[li][:])

                # z = h @ W   (node-major out)
                for c in range(NCH):
                    pA = ps.tile([128, 512], f32, name="ps", tag="ps")
                    pB = ps.tile([128, 512], f32, name="ps", tag="ps")
                    for k in range(KH):
                        lhsT = hin[k][:, c * 128:(c + 1) * 128]
                        nc.tensor.matmul(pA[:], lhsT, Wt[:, k, 0:512],
                                         start=(k == 0), stop=(k == KH - 1))
                        nc.tensor.matmul(pB[:, 0:256], lhsT, Wt[:, k, 512:768],
                                         start=(k == 0), stop=(k == KH - 1))
                    nc.vector.tensor_copy(z[:, c * H:c * H + 512], pA[:])
                    nc.vector.tensor_copy(z[:, c * H + 512:(c + 1) * H], pB[:, 0:256])

                # el/er rows
                for vec, dst_t in ((walT, elrow), (warT, errow)):
                    for half in range(2):
                        pE = ps.tile([128, 512], f32, name="ps", tag="ps")
                        for k in range(KH):
                            nc.tensor.matmul(
                                pE[0:1, :], vec[:, k:k + 1],
                                hin[k][:, half * 512:(half + 1) * 512],
                                start=(k == 0), stop=(k == KH - 1))
                        nc.vector.tensor_copy(
                            dst_t[0:1, half * 512:(half + 1) * 512], pE[0:1, :])

                if debug and li == 0:
                    nc.sync.dma_start(out=dbg_z[:], in_=z[:])

                # attention + aggregation, per dst half
                for half in range(2):
                    hs = slice(half * 512, (half + 1) * 512)
                    P = []
                    for c in range(NCH):
                        es = ps.tile([128, 512], f32, name="ps", tag="ps")
                        nc.tensor.matmul(es[:], elrow[:, c * 128:(c + 1) * 128],
                                         onesrow[:, hs], start=True, stop=False)
                        nc.tensor.matmul(es[:], onesrow[:, c * 128:(c + 1) * 128],
                                         errow[:, hs], start=False, stop=True)
                        a = abpool.tile([128, 512], bf16, name="a", tag="a")
                        nc.scalar.activation(a[:], es[:], AF.Exp)
                        b = abpool.tile([128, 512], bf16, name="b", tag="b")
                        nc.scalar.activation(b[:], es[:], AF.Exp, scale=NEG)
                        mx = abpool.tile([128, 512], bf16, name="mx", tag="mx")
                        nc.vector.tensor_max(mx[:], a[:], b[:])
                        pc = ppool.tile([128, 512], bf16, name="P", tag="P")
                        eng = nc.vector if c % 2 == 0 else nc.gpsimd
                        eng.tensor_mul(pc[:], mx[:], mm[c][:, hs])
                        P.append(pc)
                    aggp = [ps.tile([128, 512], f32, name="ps", tag="ps") for _ in range(KH)]
                    csp = ps.tile([128, 512], f32, name="ps", tag="ps")
                    for c in range(NCH):
                        for m in range(KH):
                            nc.tensor.matmul(
                                aggp[m][:], z[:, c * H + m * 128:c * H + (m + 1) * 128],
                                P[c][:], start=(c == 0), stop=(c == NCH - 1))
                        nc.tensor.matmul(csp[:], ones128[:], P[c][:],
                                         start=(c == 0), stop=(c == NCH - 1))
                    rb = npool.tile([128, 512], f32, name="rb", tag="rb")
                    nc.vector.reciprocal_approx_fast(rb[:], csp[:])
                    if debug and li == 0 and half == 0:
                        for c in range(NCH):
                            nc.sync.dma_start(out=dbg_P[c], in_=P[c][:])
                        nc.sync.dma_start(out=dbg_cs[:], in_=rb[:])
                    for m in range(KH):
                        y = npool.tile([128, 512], bf16, name="y", tag="y")
                        nc.vector.tensor_mul(y[:], aggp[m][:], rb[:])
                        e = npool.tile([128, 512], bf16, name="e", tag="e")
                        nc.scalar.activation(e[:], y[:], AF.Exp)
                        r = npool.tile([128, 512], bf16, name="r", tag="r")
                        nc.scalar.activation(r[:], y[:], AF.Relu)
                        # elu(y) = min(exp(y)-1, relu(y))
                        nc.vector.scalar_tensor_tensor(
                            hout[m][:, hs], e[:], 1.0, r[:],
                            OP.subtract, OP.min)

            if debug:
                nc.sync.dma_start(out=dbg_el[0:1, :], in_=elrow[:])
                nc.sync.dma_start(out=dbg_el[1:2, :], in_=errow[:])
                for m in range(KH):
                    nc.sync.dma_start(out=dbg_h1[m], in_=hset[1][m][:])

            # ---------------- final reduction ----------------
            h3 = hset[1]  # after 3 layers output parity = 1
            for m in range(KH):
                qfT = npool.tile([128, 1], f32, name="qfT", tag="qfT")
                nc.vector.tensor_reduce(qfT[:], qfacc[m][:], AX.X, OP.add)
                av = npool.tile([128, 1], f32, name="av", tag="av")
                nc.vector.tensor_reduce(av[:], h3[m][:], AX.X, OP.add)
                # dfin[:,m] = av/1024 - qfT  (sign irrelevant under abs)
                nc.vector.scalar_tensor_tensor(
                    dfin[:, m:m + 1], av[:], 1.0 / NPD, qfT[:],
                    OP.mult, OP.subtract)
            dfa = npool.tile([128, KH], f32, name="dfa", tag="dfa")
            nc.scalar.activation(dfa[:], dfin[:], AF.Abs)
            fin = npool.tile([1, 1], f32, name="fin", tag="fin")
            nc.gpsimd.tensor_reduce(fin[:], dfa[:], AX.XYZWC, OP.add)
            nc.sync.dma_start(out=out_d[:], in_=fin[:])

    nc.finalize()
    return nc


def _shard_inputs(inputs):
    """Host-side preprocessing: build per-core input maps."""
    import ml_dtypes
    bf = ml_dtypes.bfloat16

    f = np.asarray(inputs["features"], np.float32)
    spans = np.asarray(inputs["token_spans"])
    masks = np.asarray(inputs["masks"])
    sel = np.asarray(inputs["selected_indices"])
    src = np.asarray(inputs["src"])
    dst = np.asarray(inputs["dst"])
    doc_spans = np.asarray(inputs["doc_spans"])
    seg = np.asarray(inputs["segment_ids"])
    ish = np.asarray(inputs["is_head"])
    B = f.shape[0]

    # span/query mask matrix [B, L, SW]
    pos = np.arange(L)
    bi = np.arange(B)[:, None]
    st = spans[bi, sel, 0]          # [B, SEL]
    en = spans[bi, sel, 1]
    sm = ((pos[None, None, :] >= st[:, :, None])
          & (pos[None, None, :] < en[:, :, None])
          & (masks[:, None, :] > 0)).astype(np.float32)
    cnt = np.maximum(sm.sum(-1), 1.0)
    w = sm / cnt[:, :, None] * (en > 0).astype(np.float32)[:, :, None]  # [B,SEL,L]
    doc_cnt = np.maximum((doc_spans[:, 1] - doc_spans[:, 0]).astype(np.float32), 1.0)
    qm = (((ish != 2) & (seg == 0) & (masks > 0)).astype(np.float32))   # [B,L]
    smask_all = np.zeros((B, L, SW), np.float32)
    smask_all[:, :, :SEL] = w.transpose(0, 2, 1)

    in_maps = []
    for d in range(D):
        s0, s1 = int(doc_spans[d, 0]), int(doc_spans[d, 1])
        assert s1 - s0 == SPD, "kernel assumes 8 sentences per doc"
        sm_d = smask_all[s0:s1].copy()
        sm_d[:, :, SEL] = qm[s0:s1] / doc_cnt[d]
        f_d = f[s0:s1]
        # edges of this doc
        lo, hi = d * NPD, (d + 1) * NPD
        eidx = np.where((dst >= lo) & (dst < hi))[0]
        ls = src[eidx] - lo
        ld = dst[eidx] - lo
        assert np.all((ls >= 0) & (ls < NPD)), "edge crosses doc block"
        M = np.bincount(ls * NPD + ld, minlength=NPD * NPD).astype(np.float32)
        M = M.reshape(NPD, NPD)
        im = {
            "feats": f_d.reshape(SPD, KL, 128, H).transpose(0, 2, 1, 3).astype(bf),
            "smask": sm_d.reshape(SPD, KL, 128, SW).transpose(0, 2, 1, 3).astype(bf),
            "mmask": M.reshape(NCH, 128, NPD).astype(bf),
        }
        for i in range(3):
            W = np.asarray(inputs[f"W{i}"], np.float32)
            al = np.asarray(inputs[f"al{i}"], np.float32)
            ar = np.asarray(inputs[f"ar{i}"], np.float32)
            im[f"W{i}"] = W.reshape(KH, 128, H).transpose(1, 0, 2).astype(bf)
            im[f"wal{i}"] = (W @ al).reshape(KH, 128).T.astype(bf)
            im[f"war{i}"] = (W @ ar).reshape(KH, 128).T.astype(bf)
        in_maps.append(im)
    return in_maps


def _run(inputs, trace=False, tmpdir=None):
    _ensure_env()
    from concourse.bass_utils import run_bass_kernel_spmd
    if "nc" not in _PROG:
        _PROG["nc"] = _build_program()
    in_maps = _shard_inputs(inputs)
    res = run_bass_kernel_spmd(_PROG["nc"], in_maps, core_ids=list(range(D)),
                               trace=trace, tmpdir=tmpdir)
    out = np.array([res.results[c]["out"][0, 0] for c in range(D)], np.float32)
    return out, res


def kernel(**inputs) -> np.ndarray:
    out, _ = _run(inputs)
    return out



# revision 2
# speedup vs baseline: 1.0072x; 1.0072x over previous
"""DocGCN Trainium2 kernel v2: fp8 DoubleRow matmuls + log-mask-in-PE attention.

Per core = one doc (1024 nodes, block-diagonal graph). Layout tricks:
  - h-dims permuted s.t. sigma-chunk mi=(2m'+i) holds dims 256m'+2p+i: makes
    hT directly usable as DoubleRow (K=256) lhsT for z = h @ W.
  - P tiles [128, 2, 512] fp8e5 (sub-row i = node chunk 2c'+i) feed DoubleRow
    agg; global 2^-8 scale folded into Exp bias (cancels in normalization).
  - Edge mask folded as lnM (-200 non-edge) into ES psum via identity matmul.
  - el/er come free as extra z columns; one PE transpose + SBUF DMA makes rows.
  - elu(y) = min(exp(y)-1, relu(y)); leaky-relu via STT max(x, 0.2x).
"""

import numpy as np

SPD = 8          # sentences per doc
L = 512          # tokens per sentence
H = 768          # hidden
SEL = 128        # selected spans (graph nodes) per sentence
NPD = SPD * SEL  # nodes per doc = 1024
NCH = NPD // 128  # 8 node chunks
NSC = NCH // 2    # 4 node super-chunks (DoubleRow)
KH2 = 3          # h super-chunks of 256
SW = 144         # smask width: 128 sel cols + qmask col + pad (16B align)
WC = 784         # W' cols: 768 + el + er + pad (16B align)
D = 8            # docs = cores
NEG = 0.2
PSC = -8.0 * float(np.log(2.0))   # exp bias: global 2^-8 attention scale

_PROG = {}


def _ensure_env():
    import sys, types
    for p in ("/opt/trn_rl_repo", "/opt/trn_rl_repo/concourse"):
        if p not in sys.path:
            sys.path.insert(0, p)
    if "antenv.axon_hooks" not in sys.modules:
        try:
            import antenv
            mod = types.ModuleType("antenv.axon_hooks")
            mod._hook = None
            mod.set_axon_ntff_profile_hook = lambda h: setattr(mod, "_hook", h)
            mod.get_axon_ntff_profile_hook = lambda: mod._hook
            sys.modules["antenv.axon_hooks"] = mod
            antenv.axon_hooks = mod
            if "/root/.axon_site" not in sys.path:
                sys.path.insert(0, "/root/.axon_site")
            from trn_agent_boot import trn_boot
            h = trn_boot._ntff_profile_via_ctypes("/opt/axon/libaxon_pjrt.so")
            if h is not None:
                mod.set_axon_ntff_profile_hook(h)
        except Exception:
            pass


def _perm():
    """sigma: permuted position (2m'+i)*128+p  <-  original dim 256m'+2p+i."""
    perm = np.empty(H, np.int64)
    for mp in range(KH2):
        for i in range(2):
            for p in range(128):
                perm[(2 * mp + i) * 128 + p] = 256 * mp + 2 * p + i
    return perm


def _build_program():
    import concourse.bacc as bacc
    import concourse.tile as tile
    from concourse import mybir
    from contextlib import ExitStack

    f32 = mybir.dt.float32
    bf16 = mybir.dt.bfloat16
    f8e4 = mybir.dt.float8e4
    f8e5 = mybir.dt.float8e5
    AF = mybir.ActivationFunctionType
    OP = mybir.AluOpType
    AX = mybir.AxisListType
    DR = mybir.MatmulPerfMode.DoubleRow

    nc = bacc.Bacc(None, target_bir_lowering=False)

    fsd = nc.dram_tensor("fs", [SPD, 128, 2, 2, H + SW], f8e4,
                         kind="ExternalInput")
    lmd = nc.dram_tensor("lm", [NCH, 128, NPD], f8e4, kind="ExternalInput")
    wtd = [nc.dram_tensor(f"W{i}", [128, KH2, 2, WC], f8e4, kind="ExternalInput")
           for i in range(3)]
    idb = nc.dram_tensor("idb", [128, 128], f32, kind="ExternalInput")
    idf = nc.dram_tensor("idf", [128, 128], f8e4, kind="ExternalInput")
    out_d = nc.dram_tensor("out", [1, 1], f32, kind="ExternalOutput")
    oneer_d = nc.dram_tensor("oneer_d", [2, NPD], bf16, kind="Internal")
    zo_d = nc.dram_tensor("zo", [2, NPD], bf16, kind="ExternalInput")

    with tile.TileContext(nc) as tc:
        with ExitStack() as ctx:
            const = ctx.enter_context(tc.tile_pool(name="const", bufs=1))
            fpool = ctx.enter_context(tc.tile_pool(name="fpool", bufs=2))
            spool = ctx.enter_context(tc.tile_pool(name="spool", bufs=2))
            tpool = ctx.enter_context(tc.tile_pool(name="tpool", bufs=6))
            ppool = ctx.enter_context(tc.tile_pool(name="ppool", bufs=8))
            ypool = ctx.enter_context(tc.tile_pool(name="ypool", bufs=4))
            epool = ctx.enter_context(tc.tile_pool(name="epool", bufs=4))
            rpool = ctx.enter_context(tc.tile_pool(name="rpool", bufs=4))
            rbp = ctx.enter_context(tc.tile_pool(name="rbp", bufs=2))
            ps = ctx.enter_context(tc.tile_pool(name="ps", bufs=8, space="PSUM"))

            # ---- constants / persistent tiles ----
            identb = const.tile([128, 128], f32, name="identb", tag="identb")
            nc.sync.dma_start(out=identb[:], in_=idb[:])
            identf = const.tile([128, 128], f8e4, name="identf", tag="identf")
            nc.sync.dma_start(out=identf[:], in_=idf[:])
            wt = []
            for i in range(3):
                w = const.tile([128, KH2, 2, WC], f8e4, name=f"wt{i}", tag=f"wt{i}")
                wt.append(w)
            ones_dr = const.tile([128, 2, 128], f8e4, name="ones_dr", tag="ones_dr")
            nc.vector.memset(ones_dr[:], 1.0)
            onesrow = const.tile([1, NPD], bf16, name="onesrow", tag="onesrow")
            nc.vector.memset(onesrow[:], 1.0)
            # seed the static ones row of the DRAM bounce buffer
            nc.sync.dma_start(out=oneer_d[0:1, :], in_=onesrow[:])
            oneer = const.tile([2, NPD], bf16, name="oneer", tag="oneer")
            oneer2 = const.tile([2, NPD], bf16, name="oneer2", tag="oneer2")
            nc.sync.dma_start(out=oneer2[:], in_=zo_d[:])
            hta = const.tile([128, KH2, 2, NPD], f8e4, name="hta", tag="hta")
            htb = const.tile([128, KH2, 2, NPD], f8e4, name="htb", tag="htb")
            zt = const.tile([128, NCH, WC], f8e4, name="zt", tag="zt")
            h3s = [const.tile([128, 512], bf16, name=f"h3s{m}", tag=f"h3s{m}")
                   for m in range(2)]
            qfacc = const.tile([128, 6, SPD], f32, name="qfacc", tag="qfacc")
            avacc = const.tile([128, 12], f32, name="avacc", tag="avacc")
            elerc = const.tile([128, 16], f32, name="elerc", tag="elerc")
            elpa = const.tile([128, 8], f32, name="elpa", tag="elpa")
            elpb = const.tile([128, 8], f32, name="elpb", tag="elpb")
            errow8 = const.tile([8, 128], bf16, name="errow8", tag="errow8")
            dfin = const.tile([128, 6], f32, name="dfin", tag="dfin")
            pscb = const.tile([128, 1], f32, name="pscb", tag="pscb")
            nc.vector.memset(pscb[:], PSC)
            zeros = const.tile([128, 512], bf16, name="zeros", tag="zeros")
            nc.vector.memset(zeros[:], 0.0)
            lm = const.tile([128, NCH, NPD], f8e4, name="lm", tag="lm")

            # ---------------- helpers ----------------
            hts = [hta, htb]

            def z_chunk(li, c):
                """z = h @ W' for node chunk c (DoubleRow fp8)."""
                hin = hts[li % 2]
                W = wt[li]
                pA = ps.tile([128, 512], f32, name="ps", tag="ps")
                pB = ps.tile([128, 512], f32, name="ps", tag="ps")
                for mp in range(KH2):
                    lhsT = hin[:, mp, :, c * 128:(c + 1) * 128]
                    nc.tensor.matmul(pA[:], lhsT, W[:, mp, :, 0:512],
                                     start=(mp == 0), stop=(mp == KH2 - 1),
                                     perf_mode=DR)
                    nc.tensor.matmul(pB[:, 0:272], lhsT, W[:, mp, :, 512:WC],
                                     start=(mp == 0), stop=(mp == KH2 - 1),
                                     perf_mode=DR)
                nc.scalar.copy(zt[:, c, 0:512], pA[:])
                nc.vector.tensor_copy(zt[:, c, 512:770], pB[:, 0:258])
                nc.vector.tensor_copy(elerc[:, 2 * c:2 * c + 2],
                                      pB[:, 256:258])

            def elrow_prep():
                """er cols -> row via PE transpose + DRAM bounce; el -> exp
                bias tiles (el enters the logits as a per-partition bias)."""
                nc.vector.tensor_scalar(elpa[:], elerc[:, 0:16:2], 1.0, PSC,
                                        OP.mult, OP.add)
                nc.vector.tensor_scalar(elpb[:], elerc[:, 0:16:2], NEG, PSC,
                                        OP.mult, OP.add)
                tpB = ps.tile([128, 512], f32, name="ps", tag="ps")
                nc.tensor.transpose(tpB[0:8, 0:128], elerc[:, 1:16:2], identb[:])
                nc.vector.tensor_copy(errow8[:], tpB[0:8, 0:128])
                nc.sync.dma_start(out=oneer_d[1:2, :], in_=errow8[:])
                nc.sync.dma_start(out=oneer[:], in_=oneer_d[:])

            # ---------------- span extraction + layer-0 z ----------------
            fstiles = {}

            def fs_dma(s):
                fs = fpool.tile([128, 2, 2, H + SW], f8e4, name="fs", tag="fs")
                nc.sync.dma_start(out=fs[:], in_=fsd[s])
                fstiles[s] = fs

            fs_dma(0)
            fs_dma(1)
            for s in range(SPD):
                fs = fstiles[s]
                if s + 2 < SPD:
                    fs_dma(s + 2)
                for mi in range(6):
                    p = ps.tile([128, 512], f32, name="ps", tag="ps")
                    for k2 in range(2):
                        for i2 in range(2):
                            nc.tensor.matmul(
                                p[:, 0:129],
                                fs[:, k2, i2, mi * 128:(mi + 1) * 128],
                                fs[:, k2, i2, H:H + 129],
                                start=(k2 == 0 and i2 == 0),
                                stop=(k2 == 1 and i2 == 1))
                    if (s * 6 + mi) % 2 == 0:
                        nc.scalar.copy(
                            hta[:, mi // 2, mi % 2, s * 128:(s + 1) * 128],
                            p[:, 0:128])
                    else:
                        nc.vector.tensor_copy(
                            hta[:, mi // 2, mi % 2, s * 128:(s + 1) * 128],
                            p[:, 0:128])
                    nc.vector.tensor_copy(qfacc[:, mi, s:s + 1], p[:, 128:129])
                # stagger big constant loads behind the first feature tiles
                if s == 0:
                    nc.sync.dma_start(out=wt[0][:], in_=wtd[0][:])
                elif s == 2:
                    for c in range(4):
                        nc.sync.dma_start(out=lm[:, c, :], in_=lmd[c])
                elif s == 3:
                    for c in range(4, NCH):
                        nc.sync.dma_start(out=lm[:, c, :], in_=lmd[c])
                elif s == 4:
                    nc.sync.dma_start(out=wt[1][:], in_=wtd[1][:])
                elif s == 5:
                    nc.sync.dma_start(out=wt[2][:], in_=wtd[2][:])
                # layer-0 z for node chunk s (nodes of sentence s)
                z_chunk(0, s)

            # ---------------- GAT layers ----------------
            for li in range(3):
                hout = hts[(li + 1) % 2]
                elrow_prep()
                P = {}
                esq = {}

                def es_lm(half, c):
                    es = ps.tile([128, 512], f32, name="ps", tag="ps")
                    nc.tensor.matmul(es[:], identf[:],
                                     lm[:, c, half * 512:half * 512 + 512],
                                     start=True, stop=False)
                    esq[(half, c)] = es

                def es_fin(half, c):
                    es = esq[(half, c)]
                    nc.tensor.matmul(es[:], oneer2[:, c * 128:(c + 1) * 128],
                                     oneer[:, half * 512:half * 512 + 512],
                                     start=False, stop=True)
                    a = tpool.tile([128, 512], bf16, name="a", tag="a")
                    nc.scalar.activation(a[:], es[:], AF.Exp,
                                         bias=elpa[:, c:c + 1])
                    b = tpool.tile([128, 512], bf16, name="b", tag="b")
                    nc.scalar.activation(b[:], es[:], AF.Exp,
                                         bias=elpb[:, c:c + 1], scale=NEG)
                    if c % 2 == 0:
                        P[(half, c // 2)] = ppool.tile(
                            [128, 2, 512], f8e5, name="P", tag="P")
                    nc.vector.tensor_max(P[(half, c // 2)][:, c % 2, :],
                                         a[:], b[:])

                def csum(half, cp, csp):
                    nc.tensor.matmul(csp[:], ones_dr[:], P[(half, cp)][:],
                                     start=(cp == 0), stop=(cp == NSC - 1),
                                     perf_mode=DR)

                def recip(csp):
                    rb = rbp.tile([128, 512], f32, name="rb", tag="rb")
                    nc.vector.reciprocal_approx_fast(rb[:], csp[:])
                    return rb

                def agg_mm(half, cp, aggp):
                    for mi in range(6):
                        nc.tensor.matmul(
                            aggp[mi][:],
                            zt[:, 2 * cp:2 * cp + 2, mi * 128:(mi + 1) * 128],
                            P[(half, cp)][:],
                            start=(cp == 0), stop=(cp == NSC - 1),
                            perf_mode=DR)

                def yelu_mi(half, rb, aggt, mi):
                    y = ypool.tile([128, 512], bf16, name="y", tag="y")
                    nc.vector.tensor_mul(y[:], aggt[:], rb[:])
                    e = epool.tile([128, 512], bf16, name="e", tag="e")
                    nc.scalar.activation(e[:], y[:], AF.Exp)
                    r = rpool.tile([128, 512], bf16, name="r", tag="r")
                    if li == 2 or mi % 2 == 0:
                        nc.scalar.activation(r[:], y[:], AF.Relu)
                    else:
                        nc.vector.tensor_scalar_max(r[:], y[:], 0.0)
                    # elu = min(exp(y)-1, relu(y))
                    if li == 2:
                        dst = h3s[mi % 2][:]
                        nc.vector.scalar_tensor_tensor(
                            dst, e[:], 1.0, r[:], OP.subtract, OP.min,
                            accum_out=avacc[:, half * 6 + mi:half * 6 + mi + 1])
                    else:
                        dst = hout[:, mi // 2, mi % 2,
                                   half * 512:half * 512 + 512]
                        nc.vector.scalar_tensor_tensor(
                            dst, e[:], 1.0, r[:], OP.subtract, OP.min)

                def agg4(half, aggt, mi):
                    for cp in range(NSC):
                        nc.tensor.matmul(
                            aggt[:],
                            zt[:, 2 * cp:2 * cp + 2, mi * 128:(mi + 1) * 128],
                            P[(half, cp)][:],
                            start=(cp == 0), stop=(cp == NSC - 1),
                            perf_mode=DR)

                # half-0 P production (LM adds first to cover el/er DMA chain)
                for c in range(NCH):
                    es_lm(0, c)
                for c in range(NCH):
                    es_fin(0, c)
                csp0 = ps.tile([128, 512], f32, name="ps", tag="ps")
                for cp in range(NSC):
                    csum(0, cp, csp0)
                rb0 = recip(csp0)
                # agg half-0 (cp-outer: banks rotate, PE pipelines)
                # overlapped with half-1 P production (es double-buffered)
                aggp0 = [ps.tile([128, 512], f32, name="ps", tag="ps")
                         for _ in range(6)]
                for cp in range(NSC):
                    for mi in range(6):
                        nc.tensor.matmul(
                            aggp0[mi][:],
                            zt[:, 2 * cp:2 * cp + 2, mi * 128:(mi + 1) * 128],
                            P[(0, cp)][:],
                            start=(cp == 0), stop=(cp == NSC - 1),
                            perf_mode=DR)
                    es_lm(1, 2 * cp)
                    es_fin(1, 2 * cp)
                    es_lm(1, 2 * cp + 1)
                    es_fin(1, 2 * cp + 1)
                csp1 = ps.tile([128, 512], f32, name="ps", tag="ps")
                for cp in range(NSC):
                    csum(1, cp, csp1)
                rb1 = recip(csp1)
                for mi in range(6):
                    yelu_mi(0, rb0, aggp0[mi], mi)
                # agg half-1 overlapped with next layer's first z chunks
                aggp1 = [ps.tile([128, 512], f32, name="ps", tag="ps")
                         for _ in range(6)]
                for cp in range(NSC):
                    for mi in range(6):
                        nc.tensor.matmul(
                            aggp1[mi][:],
                            zt[:, 2 * cp:2 * cp + 2, mi * 128:(mi + 1) * 128],
                            P[(1, cp)][:],
                            start=(cp == 0), stop=(cp == NSC - 1),
                            perf_mode=DR)
                    if li < 2 and cp >= 1:
                        z_chunk(li + 1, cp - 1)
                if li < 2:
                    z_chunk(li + 1, 3)
                for mi in range(6):
                    yelu_mi(1, rb1, aggp1[mi], mi)
                if li < 2:
                    for c in range(4, NCH):
                        z_chunk(li + 1, c)

            # ---------------- final reduction (batched) ----------------
            qfT6 = rpool.tile([128, 6], f32, name="qfT6", tag="qfT6")
            nc.vector.tensor_reduce(qfT6[:], qfacc[:], AX.X, OP.add)
            u6 = rpool.tile([128, 6], f32, name="u6", tag="u6")
            nc.vector.tensor_add(u6[:], avacc[:, 0:6], avacc[:, 6:12])
            nc.vector.scalar_tensor_tensor(
                dfin[:], u6[:], 1.0 / NPD, qfT6[:], OP.mult, OP.subtract)
            dfa = rpool.tile([128, 6], f32, name="dfa", tag="dfa")
            nc.scalar.activation(dfa[:], dfin[:], AF.Abs)
            fin = rpool.tile([1, 1], f32, name="fin", tag="fin")
            nc.gpsimd.tensor_reduce(fin[:], dfa[:], AX.XYZWC, OP.add)
            nc.sync.dma_start(out=out_d[:], in_=fin[:])

    nc.finalize()
    return nc


def _shard_inputs(inputs):
    """Host-side preprocessing: build per-core input maps."""
    import ml_dtypes
    f8 = ml_dtypes.float8_e4m3
    perm = _perm()

    f = np.asarray(inputs["features"], np.float32)
    spans = np.asarray(inputs["token_spans"])
    masks = np.asarray(inputs["masks"])
    sel = np.asarray(inputs["selected_indices"])
    src = np.asarray(inputs["src"])
    dst = np.asarray(inputs["dst"])
    doc_spans = np.asarray(inputs["doc_spans"])
    seg = np.asarray(inputs["segment_ids"])
    ish = np.asarray(inputs["is_head"])
    B = f.shape[0]

    pos = np.arange(L)
    bi = np.arange(B)[:, None]
    stx = spans[bi, sel, 0]
    en = spans[bi, sel, 1]
    sm = ((pos[None, None, :] >= stx[:, :, None])
          & (pos[None, None, :] < en[:, :, None])
          & (masks[:, None, :] > 0)).astype(np.float32)
    cnt = np.maximum(sm.sum(-1), 1.0)
    w = sm / cnt[:, :, None] * (en > 0).astype(np.float32)[:, :, None]  # [B,SEL,L]
    doc_cnt = np.maximum((doc_spans[:, 1] - doc_spans[:, 0]).astype(np.float32), 1.0)
    qm = (((ish != 2) & (seg == 0) & (masks > 0)).astype(np.float32))   # [B,L]
    smask_all = np.zeros((B, L, SW), np.float32)
    smask_all[:, :, :SEL] = w.transpose(0, 2, 1)

    # shared weight tensors
    wts = []
    for i in range(3):
        W = np.asarray(inputs[f"W{i}"], np.float32)
        al = np.asarray(inputs[f"al{i}"], np.float32)
        ar = np.asarray(inputs[f"ar{i}"], np.float32)
        Wp = W[perm][:, perm]
        wal = (W @ al)[perm]
        war = (W @ ar)[perm]
        Wf = np.zeros((H, WC), np.float32)
        Wf[:, :H] = Wp
        Wf[:, H] = wal
        Wf[:, H + 1] = war
        wts.append(Wf.reshape(KH2, 2, 128, WC).transpose(2, 0, 1, 3).astype(f8))
    identb = np.eye(128, dtype=np.float32)
    identf = np.eye(128, dtype=f8)

    in_maps = []
    for d in range(D):
        s0, s1 = int(doc_spans[d, 0]), int(doc_spans[d, 1])
        assert s1 - s0 == SPD, "kernel assumes 8 sentences per doc"
        sm_d = smask_all[s0:s1].copy()
        sm_d[:, :, SEL] = qm[s0:s1] / doc_cnt[d]
        f_d = f[s0:s1][:, :, perm]                      # [SPD, L, H] permuted
        lo, hi = d * NPD, (d + 1) * NPD
        eidx = np.where((dst >= lo) & (dst < hi))[0]
        ls = src[eidx] - lo
        ld = dst[eidx] - lo
        assert np.all((ls >= 0) & (ls < NPD)), "edge crosses doc block"
        M = np.bincount(ls * NPD + ld, minlength=NPD * NPD).astype(np.float32)
        M = M.reshape(NPD, NPD)
        LM = np.where(M > 0, np.log(np.maximum(M, 1.0)), -200.0).astype(np.float32)
        fs = np.concatenate([f_d, sm_d], axis=2)     # [SPD, L, H+SW]
        im = {
            # L interleave: l = 256*k2 + 2p + i
            "fs": fs.reshape(SPD, 2, 128, 2, H + SW).transpose(0, 2, 1, 3, 4)
                    .astype(f8),
            "lm": LM.reshape(NCH, 128, NPD).astype(f8),
            "idb": identb, "idf": identf,
            "zo": np.concatenate([np.zeros((1, NPD), np.float32),
                                  np.ones((1, NPD), np.float32)]
                                 ).astype(ml_dtypes.bfloat16),
        }
        for i in range(3):
            im[f"W{i}"] = wts[i]
        in_maps.append(im)
    return in_maps


def _run(inputs, trace=False, tmpdir=None):
    _ensure_env()
    from concourse.bass_utils import run_bass_kernel_spmd
    if "nc" not in _PROG:
        _PROG["nc"] = _build_program()
    in_maps = _shard_inputs(inputs)
    res = run_bass_kernel_spmd(_PROG["nc"], in_maps, core_ids=list(range(D)),
                               trace=trace, tmpdir=tmpdir)
    out = np.array([res.results[c]["out"][0, 0] for c in range(D)], np.float32)
    return out, res


def kernel(**inputs) -> np.ndarray:
    out, _ = _run(inputs)
    return out
